# revision 9
# baseline (speedup 1.0000x reference)
"""Trainium2 Bass kernel for dense layer: out = inputs @ kernel + bias.

Shapes (hardcoded): inputs [16384, 768] f32, kernel [768, 768] f32,
bias [768] f32 -> out [16384, 768] f32.

Strategy: data-parallel over 8 NeuronCores. Each core gets a contiguous
2048-row batch slice of `inputs`; `kernel` and `bias` are replicated.
No collectives needed; outputs are concatenated on host.

Per-core kernel (Tile framework):
  - x processed in 16 tiles of 128 rows; DMA'd in groups of 2 tiles in
    natural [b, i] layout (contiguous), transposed 128x128-blockwise on
    the PE (transpose datapath) into two PSUM banks per tile, rounded
    to float32r during the (bank-wide, fused) PSUM eviction.
  - f32r matmul (1 cycle/row on PE vs 4 for f32; rel err ~1.4e-4)
    accumulates the 6 k-chunks into PSUM [128,512]+[128,256].
  - bias-add fuses with the accumulator eviction on the DVE.
  - kernel matrix W is DMA'd per k-chunk (issued after the first x
    groups so x transposes start early), rounded to f32r once.
  - PSUM: 4 pools x 2 bufs = all 8 banks.
"""

import sys

for _p in ("/opt/trn_rl_repo", "/root/.axon_site/_ro/trn_rl_repo"):
    if _p not in sys.path:
        sys.path.insert(0, _p)

import numpy as np

B, IN, UNITS = 16384, 768, 768
N_CORES = 8
B_CORE = B // N_CORES          # 2048 rows per core
P = 128
KC = IN // P                   # 6 contraction chunks
NT = B_CORE // P               # 16 row tiles per core
G = 1                          # row tiles per DMA group
N0, N1 = 512, UNITS - 512      # PSUM bank split of the 768 output cols
KC0 = N0 // P                  # 4 k-chunks land in the wide transpose bank

_cache = {}


def _build_nc():
    import concourse.mybir as mybir
    import concourse.tile as tile
    from concourse import bacc
    from concourse.masks import make_identity

    f32 = mybir.dt.float32
    f32r = mybir.dt.float32r

    nc = bacc.Bacc()
    x = nc.dram_tensor("x", [B_CORE, IN], f32, kind="ExternalInput")
    w = nc.dram_tensor("w", [IN, UNITS], f32, kind="ExternalInput")
    b = nc.dram_tensor("b", [UNITS], f32, kind="ExternalInput")
    y = nc.dram_tensor("y", [B_CORE, UNITS], f32, kind="ExternalOutput")

    x_v = x.rearrange("(g p) i -> p g i", p=P)   # row tile g, partition p
    y_v = y.rearrange("(g p) u -> p g u", p=P)
    w_v = w.rearrange("(c p) u -> p c u", p=P)   # k-chunk c, partition p

    with tile.TileContext(nc) as tc:
        with (
            tc.tile_pool(name="const", bufs=1) as const,
            tc.tile_pool(name="xin", bufs=4) as xin,
            tc.tile_pool(name="xt", bufs=3) as xt,
            tc.tile_pool(name="yout", bufs=3) as yout,
            tc.tile_pool(name="tp0", bufs=2, space="PSUM") as tp0_pool,
            tc.tile_pool(name="tp1", bufs=2, space="PSUM") as tp1_pool,
            tc.tile_pool(name="pa0", bufs=2, space="PSUM") as pa0_pool,
            tc.tile_pool(name="pa1", bufs=2, space="PSUM") as pa1_pool,
        ):
            # identity for PE-transpose (first gpsimd work of the kernel)
            ident = const.tile([P, P], f32, tag="ident")
            make_identity(nc, ident[:])

            x_bufs = {}
            def dma_x(t):
                xb = xin.tile([P, IN], f32, tag="x_buf")
                x_bufs[t] = xb
                nc.sync.dma_start(out=xb[:], in_=x_v[:, t, :])

            w_f = const.tile([P, KC, UNITS], f32, tag="w_f")
            w_r = const.tile([P, KC, UNITS], f32r, tag="w_r")
            chunk_order = list(range(KC0, KC)) + list(range(KC0))

            # startup DMA order: x0 first, then W in two transfers (the
            # [4,5] chunks the accum walk hits first, then the rest).
            dma_x(0)
            nc.sync.dma_start(out=w_f[:, KC0:KC, :], in_=w_v[:, KC0:KC, :])
            nc.vector.tensor_copy(w_r[:, KC0:KC, :], w_f[:, KC0:KC, :])
            dma_x(1)
            nc.sync.dma_start(out=w_f[:, 0:KC0, :], in_=w_v[:, 0:KC0, :])
            dma_x(2)

            bias1 = const.tile([1, UNITS], f32, tag="bias1")
            nc.sync.dma_start(out=bias1[:], in_=b[None, :])
            bias_b = const.tile([P, UNITS], f32, tag="bias_b")
            nc.gpsimd.partition_broadcast(bias_b[:], bias1[:1, :])

            def emit_transposes(t):
                """PE-transpose tile t's 6 chunks into 2 PSUM banks and
                evict per chunk (f32r rounding) in accum-walk order."""
                xn = x_bufs.pop(t)
                tp0 = tp0_pool.tile([P, N0], f32, tag="tp0")
                tp1 = tp1_pool.tile([P, N1], f32, tag="tp1")
                for c in range(KC0, KC):
                    nc.tensor.transpose(
                        tp1[:, (c - KC0) * P : (c - KC0 + 1) * P],
                        xn[:, c * P : (c + 1) * P], ident[:],
                    )
                for c in range(KC0):
                    nc.tensor.transpose(
                        tp0[:, c * P : (c + 1) * P],
                        xn[:, c * P : (c + 1) * P], ident[:],
                    )
                xt_r = xt.tile([P, KC, P], f32r, tag="xt_r")
                for c in range(KC0, KC):
                    nc.vector.tensor_copy(
                        xt_r[:, c, :], tp1[:, (c - KC0) * P : (c - KC0 + 1) * P]
                    )
                for c in range(KC0):
                    nc.vector.tensor_copy(
                        xt_r[:, c, :], tp0[:, c * P : (c + 1) * P]
                    )
                return xt_r

            def emit_accum(t, xt_r):
                p0 = pa0_pool.tile([P, N0], f32, tag="p0")
                p1 = pa1_pool.tile([P, N1], f32, tag="p1")
                for j, c in enumerate(chunk_order):
                    lhsT = xt_r[:, c, :]               # [128 i, 128 b]
                    nc.tensor.matmul(
                        p0[:], lhsT, w_r[:, c, 0:N0],
                        start=(j == 0), stop=(j == KC - 1),
                    )
                    nc.tensor.matmul(
                        p1[:], lhsT, w_r[:, c, N0:UNITS],
                        start=(j == 0), stop=(j == KC - 1),
                    )
                # bias-add eviction + per-half writeback (second half
                # doesn't wait for the first TT; shortens the tail join)
                y_buf = yout.tile([P, UNITS], f32, tag="y_buf")
                nc.vector.tensor_add(y_buf[:, 0:N0], p0[:], bias_b[:, 0:N0])
                nc.sync.dma_start(out=y_v[:, t, 0:N0], in_=y_buf[:, 0:N0])
                nc.vector.tensor_add(
                    y_buf[:, N0:UNITS], p1[:], bias_b[:, N0:UNITS]
                )
                nc.sync.dma_start(out=y_v[:, t, N0:UNITS], in_=y_buf[:, N0:UNITS])

            # software pipeline: transposes run one tile ahead of accum,
            # so each accum phase's CASTs completed during the previous
            # accum phase and the PE never waits at the tile boundary.
            xt_cur = emit_transposes(0)
            # remaining W chunks rounded after tile 0's CASTs are queued
            for c in range(KC0):
                nc.vector.tensor_copy(w_r[:, c, :], w_f[:, c, :])
            for t in range(NT):
                if t + 1 < NT:
                    xt_next = emit_transposes(t + 1)
                else:
                    xt_next = None
                emit_accum(t, xt_cur)
                xt_cur = xt_next
                ng = t + 3
                if ng < NT and ng not in x_bufs:
                    dma_x(ng)

    nc.finalize()
    return nc


def _run(inputs, kernel, bias, trace=False, **kw):
    from concourse.bass_utils import run_bass_kernel_spmd

    if "nc" not in _cache:
        _cache["nc"] = _build_nc()
    nc = _cache["nc"]

    inputs = np.ascontiguousarray(inputs, dtype=np.float32)
    kernel = np.ascontiguousarray(kernel, dtype=np.float32)
    bias = np.ascontiguousarray(bias, dtype=np.float32)

    in_maps = [
        {
            "x": inputs[c * B_CORE : (c + 1) * B_CORE],
            "w": kernel,
            "b": bias,
        }
        for c in range(N_CORES)
    ]
    res = run_bass_kernel_spmd(nc, in_maps, list(range(N_CORES)), trace=trace, **kw)
    out = np.concatenate([res.results[c]["y"] for c in range(N_CORES)], axis=0)
    return out, res


def kernel(**inputs):
    out, _ = _run(inputs["inputs"], inputs["kernel"], inputs["bias"])
    return out


# revision 11
# speedup vs baseline: 1.0399x; 1.0399x over previous
"""Trainium2 Bass kernel for dense layer: out = inputs @ kernel + bias.

Shapes (hardcoded): inputs [16384, 768] f32, kernel [768, 768] f32,
bias [768] f32 -> out [16384, 768] f32.

Strategy: data-parallel over 8 NeuronCores. Each core gets a contiguous
2048-row batch slice of `inputs`; `kernel` and `bias` are replicated.
No collectives needed; outputs are concatenated on host.

Per-core kernel (Tile framework):
  - x processed in 16 tiles of 128 rows; DMA'd in groups of 2 tiles in
    natural [b, i] layout (contiguous), transposed 128x128-blockwise on
    the PE (transpose datapath) into two PSUM banks per tile, rounded
    to float32r during the (bank-wide, fused) PSUM eviction.
  - f32r matmul (1 cycle/row on PE vs 4 for f32; rel err ~1.4e-4)
    accumulates the 6 k-chunks into PSUM [128,512]+[128,256].
  - bias-add fuses with the accumulator eviction on the DVE.
  - kernel matrix W is DMA'd per k-chunk (issued after the first x
    groups so x transposes start early), rounded to f32r once.
  - PSUM: 4 pools x 2 bufs = all 8 banks.
"""

import sys

for _p in ("/opt/trn_rl_repo", "/root/.axon_site/_ro/trn_rl_repo"):
    if _p not in sys.path:
        sys.path.insert(0, _p)

import numpy as np

B, IN, UNITS = 16384, 768, 768
N_CORES = 8
B_CORE = B // N_CORES          # 2048 rows per core
P = 128
KC = IN // P                   # 6 contraction chunks
NT = B_CORE // P               # 16 row tiles per core
G = 1                          # row tiles per DMA group
N0, N1 = 512, UNITS - 512      # PSUM bank split of the 768 output cols
KC0 = N0 // P                  # 4 k-chunks land in the wide transpose bank

_cache = {}


def _build_nc():
    import concourse.mybir as mybir
    import concourse.tile as tile
    from concourse import bacc
    from concourse.masks import make_identity

    f32 = mybir.dt.float32
    f32r = mybir.dt.float32r

    nc = bacc.Bacc()
    x = nc.dram_tensor("x", [B_CORE, IN], f32, kind="ExternalInput")
    w = nc.dram_tensor("w", [IN, UNITS], f32, kind="ExternalInput")
    b = nc.dram_tensor("b", [UNITS], f32, kind="ExternalInput")
    y = nc.dram_tensor("y", [B_CORE, UNITS], f32, kind="ExternalOutput")

    x_v = x.rearrange("(g p) i -> p g i", p=P)   # row tile g, partition p
    y_v = y.rearrange("(g p) u -> p g u", p=P)
    w_v = w.rearrange("(c p) u -> p c u", p=P)   # k-chunk c, partition p

    with tile.TileContext(nc) as tc:
        with (
            tc.tile_pool(name="const", bufs=1) as const,
            tc.tile_pool(name="xin", bufs=6) as xin,
            tc.tile_pool(name="xt", bufs=4) as xt,
            tc.tile_pool(name="yout", bufs=3) as yout,
            tc.tile_pool(name="tp0", bufs=2, space="PSUM") as tp0_pool,
            tc.tile_pool(name="tp1", bufs=2, space="PSUM") as tp1_pool,
            tc.tile_pool(name="pa0", bufs=2, space="PSUM") as pa0_pool,
            tc.tile_pool(name="pa1", bufs=2, space="PSUM") as pa1_pool,
        ):
            # identity for PE-transpose (first gpsimd work of the kernel)
            ident = const.tile([P, P], f32, tag="ident")
            make_identity(nc, ident[:])

            x_bufs = {}
            def dma_x(t):
                xb = xin.tile([P, IN], f32, tag="x_buf")
                x_bufs[t] = xb
                nc.sync.dma_start(out=xb[:], in_=x_v[:, t, :])

            w_f = const.tile([P, KC, UNITS], f32, tag="w_f")
            w_r = const.tile([P, KC, UNITS], f32r, tag="w_r")
            chunk_order = list(range(KC0, KC)) + list(range(KC0))

            # startup DMA order: x0 first, then W in two transfers (the
            # [4,5] chunks the accum walk hits first, then the rest).
            dma_x(0)
            nc.sync.dma_start(out=w_f[:, KC0:KC, :], in_=w_v[:, KC0:KC, :])
            nc.vector.tensor_copy(w_r[:, KC0:KC, :], w_f[:, KC0:KC, :])
            dma_x(1)
            nc.sync.dma_start(out=w_f[:, 0:KC0, :], in_=w_v[:, 0:KC0, :])
            dma_x(2)
            dma_x(3)

            bias1 = const.tile([1, UNITS], f32, tag="bias1")
            nc.sync.dma_start(out=bias1[:], in_=b[None, :])
            bias_b = const.tile([P, UNITS], f32, tag="bias_b")
            nc.gpsimd.partition_broadcast(bias_b[:], bias1[:1, :])

            def emit_transposes(t):
                """PE-transpose tile t's 6 chunks into 2 PSUM banks and
                evict per chunk (f32r rounding) in accum-walk order."""
                xn = x_bufs.pop(t)
                tp0 = tp0_pool.tile([P, N0], f32, tag="tp0")
                tp1 = tp1_pool.tile([P, N1], f32, tag="tp1")
                for c in range(KC0, KC):
                    nc.tensor.transpose(
                        tp1[:, (c - KC0) * P : (c - KC0 + 1) * P],
                        xn[:, c * P : (c + 1) * P], ident[:],
                    )
                for c in range(KC0):
                    nc.tensor.transpose(
                        tp0[:, c * P : (c + 1) * P],
                        xn[:, c * P : (c + 1) * P], ident[:],
                    )
                xt_r = xt.tile([P, KC, P], f32r, tag="xt_r")
                for c in range(KC0, KC):
                    nc.vector.tensor_copy(
                        xt_r[:, c, :], tp1[:, (c - KC0) * P : (c - KC0 + 1) * P]
                    )
                for c in range(KC0):
                    nc.vector.tensor_copy(
                        xt_r[:, c, :], tp0[:, c * P : (c + 1) * P]
                    )
                return xt_r

            def emit_accum(t, xt_r):
                p0 = pa0_pool.tile([P, N0], f32, tag="p0")
                p1 = pa1_pool.tile([P, N1], f32, tag="p1")
                for j, c in enumerate(chunk_order):
                    lhsT = xt_r[:, c, :]               # [128 i, 128 b]
                    nc.tensor.matmul(
                        p0[:], lhsT, w_r[:, c, 0:N0],
                        start=(j == 0), stop=(j == KC - 1),
                    )
                    nc.tensor.matmul(
                        p1[:], lhsT, w_r[:, c, N0:UNITS],
                        start=(j == 0), stop=(j == KC - 1),
                    )
                # bias-add eviction + per-half writeback (second half
                # doesn't wait for the first TT; shortens the tail join)
                y_buf = yout.tile([P, UNITS], f32, tag="y_buf")
                nc.vector.tensor_add(y_buf[:, 0:N0], p0[:], bias_b[:, 0:N0])
                nc.sync.dma_start(out=y_v[:, t, 0:N0], in_=y_buf[:, 0:N0])
                nc.vector.tensor_add(
                    y_buf[:, N0:UNITS], p1[:], bias_b[:, N0:UNITS]
                )
                nc.sync.dma_start(out=y_v[:, t, N0:UNITS], in_=y_buf[:, N0:UNITS])

            # software pipeline, depth 2: transposes run two tiles ahead
            # of accum, so each accum phase's CASTs completed well before
            # the PE needs them, and the early W-DMA wait is filled with
            # transpose work instead of PE idle.
            xts = {0: emit_transposes(0)}
            # remaining W chunks rounded after tile 0's CASTs are queued
            for c in range(KC0):
                nc.vector.tensor_copy(w_r[:, c, :], w_f[:, c, :])
            xts[1] = emit_transposes(1)
            for t in range(NT):
                if t + 2 < NT:
                    xts[t + 2] = emit_transposes(t + 2)
                emit_accum(t, xts.pop(t))
                ng = t + 4
                if ng < NT and ng not in x_bufs:
                    dma_x(ng)

    nc.finalize()
    return nc


def _run(inputs, kernel, bias, trace=False, **kw):
    from concourse.bass_utils import run_bass_kernel_spmd

    if "nc" not in _cache:
        _cache["nc"] = _build_nc()
    nc = _cache["nc"]

    inputs = np.ascontiguousarray(inputs, dtype=np.float32)
    kernel = np.ascontiguousarray(kernel, dtype=np.float32)
    bias = np.ascontiguousarray(bias, dtype=np.float32)

    in_maps = [
        {
            "x": inputs[c * B_CORE : (c + 1) * B_CORE],
            "w": kernel,
            "b": bias,
        }
        for c in range(N_CORES)
    ]
    res = run_bass_kernel_spmd(nc, in_maps, list(range(N_CORES)), trace=trace, **kw)
    out = np.concatenate([res.results[c]["y"] for c in range(N_CORES)], axis=0)
    return out, res


def kernel(**inputs):
    out, _ = _run(inputs["inputs"], inputs["kernel"], inputs["bias"])
    return out
